# revision 81
# baseline (speedup 1.0000x reference)
"""Sharded multi-head attention for TRN2 (8 NeuronCores).

Problem: B=4, H=16, S=2048, DK=64 attention with boolean mask [B,1,S,S]
(True entries masked out).  The 64 (batch, head) pairs are independent:
core c handles batch c//2, heads (c%2)*8 .. (c%2)*8+8.

Design (v2, 343us -> 252us): single stream, the exp is split between the
scalar and vector engines, and softmax normalization happens on the HOST.
All three compute engines run ~85% busy; the scalar engine's exp
throughput (1 elem/cycle/lane @1.2GHz, ~260ns fixed cost per ACTIVATE)
is the fundamental limiter.

  - Iteration (qc-block, pr, kt): one slot = scores sc [128, 2 heads,
    512 q] f32 in PSUM (k on partitions).  Masks are reused by all 4
    head-pairs inside a 64-slot block, so the early DMA burst is 4x
    smaller than a pr-outer order.
  - exp split by q column: the first QA=352 columns of each head go
    through the scalar engine (exact exp, then a multiplicative bf16
    keep-mask tensor_tensor on DVE at 2x over both slots of a pair);
    the remaining QD=160 go through ONE DVE scalar_tensor_tensor:
    u16 = sat_u16(round(s * (128/ln2)/8 + maskadd)), maskadd = +16256
    (=127<<7, bf16-exact) kept / -65536 masked.  The u16 bits viewed
    as bf16 ARE exp(s/8) (Schraudolph); negative saturation makes
    masked weights exactly +0.0, fusing the mask into the same pass.
    The bit-trick's sawtooth error is common-mode-cancelled by softmax
    (measured end-to-end 9.1e-3 vs 3.4e-3 all-exact; gate is 2e-2).
  - PSUM: sc rotates over THREE 2-bank tags (QK is issued 2 slots
    ahead and never waits on the exp of the tile it replaces) + one
    2-bank acc [65, 2 heads, 512] (each head's f32 row fits one bank
    exactly, so matmul start=True bank-clears stay confined).
  - PV per (kt, head) is two matmuls (ACT-part columns from the bf16
    wa tile, DVE-part columns from the u16 wd tile bitcast to bf16)
    accumulating into disjoint column ranges of the same acc bank;
    only the first piece uses start=True (a start clears has_written
    for the whole bank; the second piece's first write lands on
    cleared bits = overwrite, a free region reset).  Row 64
    accumulates the softmax denominator via the ones column in vp.
    Pops lag issue by 4+ slots so PV never waits on the DVE mask op,
    and pops are issued before the slot's exp so the per-block acc
    staging copy (scalar engine) jumps ahead of it.
  - When a (pr, qc) finishes, acc is copied PSUM->SBUF (scalar
    engine; DMA cannot touch PSUM) and DMA'd out raw; the host does
    num/den.  No on-device reciprocal/normalize at all.
  - Separate wa/wd tiles per slot-pair/slot: writes from different
    engines never share a tile (the dependency tracker is
    tile-granular; sharing serializes the writers).
  - DMAs: inputs on the sync + scalar(prologue-only) HWDGE rings in
    exact consumption order; vp on the gpsimd SWDGE queue; outputs on
    sync (the last block's output is split across sync + scalar rings
    to shorten the tail).
"""

import numpy as np
import ml_dtypes
from contextlib import ExitStack

import concourse.tile as tile
from concourse import bacc, mybir
from concourse.bass_utils import run_bass_kernel_spmd

B, H, S, DK = 4, 16, 2048, 64
N_CORES = 8
HPC = (B * H) // N_CORES  # heads per core = 8
NPAIR = HPC // 2

P = 128            # k-tile size / partition count
NKT = S // P       # 16 k tiles
QCH = 512          # q chunk per head (pair tile = [128, 1024] = 2 PSUM banks)
NQ = S // QCH      # 4 q chunks

QA = 352           # q columns per head through scalar-engine exp
QD = QCH - QA      # q columns per head through DVE bit-trick exp

BF16 = mybir.dt.bfloat16
F32 = mybir.dt.float32
U16 = mybir.dt.uint16
BF = ml_dtypes.bfloat16

LN2 = float(np.log(2.0))
SCHRAU_SCALE = 128.0 / LN2 * 0.125   # folds the 1/sqrt(dk)=1/8 score scale
MADD_KEEP = 127.0 * 128.0            # 16256, exactly representable in bf16
MADD_MASK = -65536.0                 # saturates u16 convert to 0 -> +0.0 bf16

PV_LAG = 4  # pops trail far enough that PV never waits on the DVE mask op


def build_nc():
    nc = bacc.Bacc(None, target_bir_lowering=False)
    # qkt[pair, 0] = [Q_A^T ; Q_B^T] stacked on partitions, [pair, 1] = K
    qkt_ext = nc.declare_dram_parameter("qkt", [NPAIR, 2, P, S], BF16, isOutput=False)
    # vp[h, p, t, :] = [V[h, t*128+p, :], 1.0]
    vp_ext = nc.declare_dram_parameter("vp", [HPC, P, NKT, DK + 1], BF16, isOutput=False)
    # keep_act[p, qc, t, 0:QA] = not mask[qc*512+j, t*128+p]   (ACT part)
    keep_ext = nc.declare_dram_parameter("keepa", [P, NQ, NKT, QA], BF16, isOutput=False)
    # maskadd[p, qc, t, 0:QD]: +16256 kept / -65536 masked       (DVE part)
    madd_ext = nc.declare_dram_parameter("madd", [P, NQ, NKT, QD], BF16, isOutput=False)
    # raw acc dump: [pr, qc, 65, 2, 512]; host computes rows0:64 / row64
    out_ext = nc.declare_dram_parameter(
        "outT", [NPAIR, NQ, DK + 1, 2, QCH], F32, isOutput=True
    )

    with tile.TileContext(nc) as tc, ExitStack() as ctx:
        singles = ctx.enter_context(tc.tile_pool(name="singles", bufs=1))
        w_pool = ctx.enter_context(tc.tile_pool(name="wp", bufs=8))
        wd_pool = ctx.enter_context(tc.tile_pool(name="wdp", bufs=10))
        ep_pool = ctx.enter_context(tc.tile_pool(name="ep", bufs=4))
        ps_pool = ctx.enter_context(tc.tile_pool(name="ps", bufs=1, space="PSUM"))

        # ---- persistent SBUF tiles ----
        qT, kT, vpt = {}, {}, {}
        for pr in range(NPAIR):
            qT[pr] = singles.tile([P, S], BF16, name=f"qT{pr}")
            kT[pr] = singles.tile([P, S], BF16, name=f"kT{pr}")
        for h in range(HPC):
            vpt[h] = singles.tile([P, NKT, DK + 1], BF16, name=f"vph{h}")
        keep_sb = singles.tile([P, NQ, NKT, QA], BF16, name="keep_sb")
        madd_sb = singles.tile([P, NQ, NKT, QD], BF16, name="madd_sb")
        dummy = singles.tile([1, 2], F32, name="dummy")
        nc.gpsimd.memset(dummy, 0.0)

        # ---- input DMAs, in consumption order, spread over queues ----
        # iteration is (qc-block, pr, kt): the whole first block (64 slots)
        # uses only qc0 masks, qc0 columns of qT, all of kT, all vp.
        # act-ring DMAs are prologue-only (its sequencer idles pre-exp).
        nc.scalar.dma_start(out=qT[0][:, 0:QCH], in_=qkt_ext[0, 0, :, 0:QCH])
        # warm the ACT exp table while the DMAs fill
        nc.scalar.activation(
            dummy, dummy, mybir.ActivationFunctionType.Exp, scale=1.0
        )
        nc.scalar.dma_start(out=keep_sb[:, 0, 0:4], in_=keep_ext[:, 0, 0:4])
        nc.scalar.dma_start(out=keep_sb[:, 0, 4:8], in_=keep_ext[:, 0, 4:8])
        # sync ring: interleave kT0 / qc0 madd slices in consumption order
        # (slot k of block 0 needs kT0[:, 128k:...] and madd[qc0, k])
        nc.sync.dma_start(out=kT[0][:, 0:P], in_=qkt_ext[0, 1, :, 0:P])
        nc.sync.dma_start(out=madd_sb[:, 0, 0:2], in_=madd_ext[:, 0, 0:2])
        nc.sync.dma_start(out=kT[0][:, P : 4 * P], in_=qkt_ext[0, 1, :, P : 4 * P])
        nc.sync.dma_start(out=madd_sb[:, 0, 2:6], in_=madd_ext[:, 0, 2:6])
        nc.sync.dma_start(out=kT[0][:, 4 * P : 8 * P], in_=qkt_ext[0, 1, :, 4 * P : 8 * P])
        nc.sync.dma_start(out=madd_sb[:, 0, 6:NKT], in_=madd_ext[:, 0, 6:NKT])
        nc.sync.dma_start(out=kT[0][:, 8 * P : S], in_=qkt_ext[0, 1, :, 8 * P : S])
        nc.sync.dma_start(out=keep_sb[:, 0, 8:12], in_=keep_ext[:, 0, 8:12])
        nc.sync.dma_start(out=keep_sb[:, 0, 12:NKT], in_=keep_ext[:, 0, 12:NKT])
        for pr in (1, 2, 3):
            nc.sync.dma_start(out=kT[pr], in_=qkt_ext[pr, 1])
            nc.sync.dma_start(out=qT[pr][:, 0:QCH], in_=qkt_ext[pr, 0, :, 0:QCH])
        # later q-chunks + masks per qc block, in consumption order
        for qc in range(1, NQ):
            q0 = qc * QCH
            for pr in range(NPAIR):
                nc.sync.dma_start(
                    out=qT[pr][:, q0 : q0 + QCH],
                    in_=qkt_ext[pr, 0, :, q0 : q0 + QCH],
                )
            nc.sync.dma_start(out=madd_sb[:, qc], in_=madd_ext[:, qc])
            nc.sync.dma_start(out=keep_sb[:, qc, 0:8], in_=keep_ext[:, qc, 0:8])
            nc.sync.dma_start(out=keep_sb[:, qc, 8:NKT], in_=keep_ext[:, qc, 8:NKT])
        # vp on the gpsimd SWDGE queue, per head pair (pair pr from slot 16*pr)
        nc.gpsimd.dma_start(out=vpt[0][:, 0:4], in_=vp_ext[0, :, 0:4])
        nc.gpsimd.dma_start(out=vpt[1][:, 0:4], in_=vp_ext[1, :, 0:4])
        nc.gpsimd.dma_start(out=vpt[0][:, 4:NKT], in_=vp_ext[0, :, 4:NKT])
        nc.gpsimd.dma_start(out=vpt[1][:, 4:NKT], in_=vp_ext[1, :, 4:NKT])
        for h in (2, 3, 4, 5, 6, 7):
            nc.gpsimd.dma_start(out=vpt[h], in_=vp_ext[h])

        iters = [
            (pr, qc, kt)
            for qc in range(NQ)
            for pr in range(NPAIR)
            for kt in range(NKT)
        ]
        NSLOT = len(iters)  # 256

        st = {"acc": None, "pend": [], "sc": {}}

        def issue_qk(i):
            pr, qc, kt = iters[i]
            q0, k0 = qc * QCH, kt * P
            sc = ps_pool.tile(
                [P, 2, QCH], F32, tag=f"sc{i % 3}", name=f"sc_{i}", bufs=1
            )
            st["sc"][i] = sc
            nc.tensor.matmul(
                sc[:, 0],
                kT[pr][0:DK, k0 : k0 + P],
                qT[pr][0:DK, q0 : q0 + QCH],
                start=True, stop=True, tile_position=(0, 0),
            )
            nc.tensor.matmul(
                sc[:, 1],
                kT[pr][DK : 2 * DK, k0 : k0 + P],
                qT[pr][DK : 2 * DK, q0 : q0 + QCH],
                start=True, stop=True, tile_position=(64, 0),
            )

        def issue_pv(ent):
            kt, pr, qc, wa, wd, acc, vA, vB = ent
            st_, sp = (kt == 0), (kt == NKT - 1)
            # acc is [65, 2, 512]: head h confined to its own PSUM bank.
            # start=True clears has_written for the WHOLE bank, so only the
            # first piece may use it; the D piece's first write then lands on
            # cleared bits, which means overwrite -- a free region reset.
            # long streams first so the next LDWEIGHTS hides under them
            nc.tensor.matmul(
                acc[:, 0, 0:QA], vA[:, kt], wa[:, 0], start=st_, stop=sp
            )
            nc.tensor.matmul(
                acc[:, 1, 0:QA], vB[:, kt], wa[:, 1], start=st_, stop=sp
            )
            nc.tensor.matmul(
                acc[:, 0, QA:QCH], vA[:, kt], wd[:, 0].bitcast(BF16),
                start=False, stop=sp, skip_group_check=True,
            )
            nc.tensor.matmul(
                acc[:, 1, QA:QCH], vB[:, kt], wd[:, 1].bitcast(BF16),
                start=False, stop=sp, skip_group_check=True,
            )
            if kt == NKT - 1:
                # (pr, qc) complete: stage both heads' accs in SBUF with one
                # scalar-engine copy (frees the banks), DMA out; host divides.
                accS = ep_pool.tile(
                    [DK + 1, 2, QCH], F32, tag="accS", name=f"accS_{pr}_{qc}"
                )
                if pr == NPAIR - 1 and qc == NQ - 1:
                    # last block: split the copy + DMA so the tail overlaps
                    nc.scalar.copy(accS[:, 0], acc[:, 0])
                    nc.sync.dma_start(
                        out=out_ext[pr, qc, :, 0], in_=accS[:, 0]
                    )
                    nc.scalar.copy(accS[:, 1], acc[:, 1])
                    nc.scalar.dma_start(
                        out=out_ext[pr, qc, :, 1], in_=accS[:, 1]
                    )
                else:
                    nc.scalar.copy(accS, acc)
                    nc.sync.dma_start(out=out_ext[pr, qc], in_=accS)

        def run_slot(s):
            pr, qc, kt = iters[s]
            if kt == 0:
                st["acc"] = ps_pool.tile(
                    [DK + 1, 2, QCH], F32, tag="accAB",
                    name=f"acc_{pr}_{qc}", bufs=1,
                )
            if s % 2 == 0:
                # ACT-part weights for this slot pair: [p, slot, head, QA]
                st["wa"] = w_pool.tile(
                    [P, 2, 2, QA], BF16, tag="wa", name=f"wa_{s}"
                )
            wd = wd_pool.tile([P, 2, QD], U16, tag="wd", name=f"wd_{s}")
            # lagged PV pop: strictly one per slot in steady state
            # double-pop late in each 16-slot block so the block's last PVs
            # (and the acc staging copy behind them) run with extra lead
            # before the next block's first PV needs the acc banks
            npop = 2 if (s % 16 >= 14 or s >= NSLOT - 8) else 1
            for _ in range(npop):
                if len(st["pend"]) <= PV_LAG:
                    break
                issue_pv(st["pend"].pop(0))

            sc = st["sc"].pop(s)
            nc.scalar.activation(
                st["wa"][:, s % 2],
                sc[:, :, 0:QA],
                mybir.ActivationFunctionType.Exp,
                scale=0.125,
            )
            madd_ap = (
                madd_sb[:, qc, kt, :].unsqueeze(1).to_broadcast([P, 2, QD])
            )
            nc.vector.scalar_tensor_tensor(
                wd,
                sc[:, :, QA:QCH],
                SCHRAU_SCALE,
                madd_ap,
                mybir.AluOpType.mult,
                mybir.AluOpType.add,
            )
            if s + 2 < NSLOT:
                issue_qk(s + 2)

            if s % 2 == 1:
                # multiplicative keep mask over BOTH slots' ACT columns
                keep4 = (
                    keep_sb[:, qc, kt - 1 : kt + 1, :]
                    .unsqueeze(2)
                    .to_broadcast([P, 2, 2, QA])
                )
                nc.vector.tensor_mul(st["wa"], st["wa"], keep4)
                hA, hB = 2 * pr, 2 * pr + 1
                st["pend"].append(
                    (kt - 1, pr, qc, st["wa"][:, 0], st["wd_even"],
                     st["acc"], vpt[hA], vpt[hB])
                )
                st["pend"].append(
                    (kt, pr, qc, st["wa"][:, 1], wd,
                     st["acc"], vpt[hA], vpt[hB])
                )
            else:
                st["wd_even"] = wd
        issue_qk(0)
        issue_qk(1)
        for s in range(NSLOT):
            run_slot(s)
        while st["pend"]:
            issue_pv(st["pend"].pop(0))
    nc.finalize()
    return nc


_NC_CACHE = {}


def get_nc():
    if "nc" not in _NC_CACHE:
        _NC_CACHE["nc"] = build_nc()
    return _NC_CACHE["nc"]


def kernel(Q, K, V, mask, _trace=False, _tmpdir=None):
    Q = np.asarray(Q, dtype=np.float32)
    K = np.asarray(K, dtype=np.float32)
    V = np.asarray(V, dtype=np.float32)
    mask = np.asarray(mask)

    in_maps = []
    for c in range(N_CORES):
        b, h0 = c // 2, (c % 2) * HPC
        # [pair, {q,k}, 128, S]: partitions 0:64 = head A dims, 64:128 = head B
        qkt = np.empty((NPAIR, 2, P, S), BF)
        qt = Q[b, h0 : h0 + HPC].transpose(0, 2, 1).reshape(NPAIR, 2 * DK, S)
        kt = K[b, h0 : h0 + HPC].transpose(0, 2, 1).reshape(NPAIR, 2 * DK, S)
        qkt[:, 0] = qt
        qkt[:, 1] = kt
        vp = np.empty((HPC, P, NKT, DK + 1), BF)
        vp[:, :, :, 0:DK] = (
            V[b, h0 : h0 + HPC].reshape(HPC, NKT, P, DK).transpose(0, 2, 1, 3)
        )
        vp[:, :, :, DK] = 1.0
        if c % 2 == 0:
            # kp[k, q] = not mask[q, k]; layout [p, qc, kt, q-within-chunk]
            kp = (~mask[b, 0]).T.reshape(NKT, P, NQ, QCH).transpose(1, 2, 0, 3)
            keepa = np.ascontiguousarray(kp[:, :, :, 0:QA]).astype(BF)
            madd = np.where(
                kp[:, :, :, QA:QCH], MADD_KEEP, MADD_MASK
            ).astype(BF)
            madd = np.ascontiguousarray(madd)
        in_maps.append({"qkt": qkt, "vp": vp, "keepa": keepa, "madd": madd})

    nc = get_nc()
    res = run_bass_kernel_spmd(
        nc, in_maps, core_ids=list(range(N_CORES)), trace=_trace, tmpdir=_tmpdir
    )
    out = np.empty((B, H, S, DK), np.float32)
    for c in range(N_CORES):
        b, h0 = c // 2, (c % 2) * HPC
        acc = np.asarray(res.results[c]["outT"])  # [pr, qc, DK+1, 2, QCH]
        num = acc[:, :, 0:DK]                     # [pr, qc, d, h2, q]
        den = acc[:, :, DK]                       # [pr, qc, h2, q]
        o = num / den[:, :, None, :, :]
        # [pr, qc, d, h2, q] -> [pr, h2, qc, q, d] -> [h, q_all, d]
        out[b, h0 : h0 + HPC] = (
            o.transpose(0, 3, 1, 4, 2).reshape(HPC, S, DK)
        )
    if _trace:
        return out, res
    return out


# revision 82
# speedup vs baseline: 1.1799x; 1.1799x over previous
"""Sharded multi-head attention for TRN2 (8 NeuronCores).

Problem: B=4, H=16, S=2048, DK=64 attention with boolean mask [B,1,S,S]
(True entries masked out).  The 64 (batch, head) pairs are independent:
core c handles batch c//2, heads (c%2)*8 .. (c%2)*8+8.

Design (v2, 343us -> 252us): single stream, the exp is split between the
scalar and vector engines, and softmax normalization happens on the HOST.
All three compute engines run ~85% busy; the scalar engine's exp
throughput (1 elem/cycle/lane @1.2GHz, ~260ns fixed cost per ACTIVATE)
is the fundamental limiter.

  - Iteration (qc-block, pr, kt): one slot = scores sc [128, 2 heads,
    512 q] f32 in PSUM (k on partitions).  Masks are reused by all 4
    head-pairs inside a 64-slot block, so the early DMA burst is 4x
    smaller than a pr-outer order.
  - exp split by q column: the first QA=352 columns of each head go
    through the scalar engine (exact exp, then a multiplicative bf16
    keep-mask tensor_tensor on DVE at 2x over both slots of a pair);
    the remaining QD=160 go through ONE DVE scalar_tensor_tensor:
    u16 = sat_u16(round(s * (128/ln2)/8 + maskadd)), maskadd = +16256
    (=127<<7, bf16-exact) kept / -65536 masked.  The u16 bits viewed
    as bf16 ARE exp(s/8) (Schraudolph); negative saturation makes
    masked weights exactly +0.0, fusing the mask into the same pass.
    The bit-trick's sawtooth error is common-mode-cancelled by softmax
    (measured end-to-end 9.1e-3 vs 3.4e-3 all-exact; gate is 2e-2).
  - PSUM: sc rotates over THREE 2-bank tags (QK is issued 2 slots
    ahead and never waits on the exp of the tile it replaces) + one
    2-bank acc [65, 2 heads, 512] (each head's f32 row fits one bank
    exactly, so matmul start=True bank-clears stay confined).
  - PV per (kt, head) is two matmuls (ACT-part columns from the bf16
    wa tile, DVE-part columns from the u16 wd tile bitcast to bf16)
    accumulating into disjoint column ranges of the same acc bank;
    only the first piece uses start=True (a start clears has_written
    for the whole bank; the second piece's first write lands on
    cleared bits = overwrite, a free region reset).  Row 64
    accumulates the softmax denominator via the ones column in vp.
    Pops lag issue by 4+ slots so PV never waits on the DVE mask op,
    and pops are issued before the slot's exp so the per-block acc
    staging copy (scalar engine) jumps ahead of it.
  - When a (pr, qc) finishes, acc is copied PSUM->SBUF (scalar
    engine; DMA cannot touch PSUM) and DMA'd out raw; the host does
    num/den.  No on-device reciprocal/normalize at all.
  - Separate wa/wd tiles per slot-pair/slot: writes from different
    engines never share a tile (the dependency tracker is
    tile-granular; sharing serializes the writers).
  - DMAs: inputs on the sync + scalar(prologue-only) HWDGE rings in
    exact consumption order; vp on the gpsimd SWDGE queue; outputs on
    sync (the last block's output is split across sync + scalar rings
    to shorten the tail).
"""

import numpy as np
import ml_dtypes
from contextlib import ExitStack

import concourse.tile as tile
from concourse import bacc, mybir
from concourse.bass_utils import run_bass_kernel_spmd

B, H, S, DK = 4, 16, 2048, 64
N_CORES = 8
HPC = (B * H) // N_CORES  # heads per core = 8
NPAIR = HPC // 2

P = 128            # k-tile size / partition count
NKT = S // P       # 16 k tiles
QCH = 512          # q chunk per head (pair tile = [128, 1024] = 2 PSUM banks)
NQ = S // QCH      # 4 q chunks

QA = 352           # q columns per head through scalar-engine exp
QD = QCH - QA      # q columns per head through DVE bit-trick exp

BF16 = mybir.dt.bfloat16
F32 = mybir.dt.float32
U16 = mybir.dt.uint16
BF = ml_dtypes.bfloat16

LN2 = float(np.log(2.0))
SCHRAU_SCALE = 128.0 / LN2 * 0.125   # folds the 1/sqrt(dk)=1/8 score scale
MADD_KEEP = 127.0 * 128.0            # 16256, exactly representable in bf16
MADD_MASK = -65536.0                 # saturates u16 convert to 0 -> +0.0 bf16

PV_LAG = 4  # pops trail far enough that PV never waits on the DVE mask op


def build_nc():
    nc = bacc.Bacc(None, target_bir_lowering=False)
    # qkt[pair, 0] = [Q_A^T ; Q_B^T] stacked on partitions, [pair, 1] = K
    qkt_ext = nc.declare_dram_parameter("qkt", [NPAIR, 2, P, S], BF16, isOutput=False)
    # vp[h, p, t, :] = [V[h, t*128+p, :], 1.0]
    vp_ext = nc.declare_dram_parameter("vp", [HPC, P, NKT, DK + 1], BF16, isOutput=False)
    # keep_act[p, qc, t, 0:QA] = not mask[qc*512+j, t*128+p]   (ACT part)
    keep_ext = nc.declare_dram_parameter("keepa", [P, NQ, NKT, QA], BF16, isOutput=False)
    # maskadd[p, qc, t, 0:QD]: +16256 kept / -65536 masked       (DVE part)
    madd_ext = nc.declare_dram_parameter("madd", [P, NQ, NKT, QD], BF16, isOutput=False)
    # raw acc dump: [pr, qc, 65, 2, 512]; host computes rows0:64 / row64
    out_ext = nc.declare_dram_parameter(
        "outT", [NPAIR, NQ, DK + 1, 2, QCH], F32, isOutput=True
    )

    with tile.TileContext(nc) as tc, ExitStack() as ctx:
        singles = ctx.enter_context(tc.tile_pool(name="singles", bufs=1))
        w_pool = ctx.enter_context(tc.tile_pool(name="wp", bufs=8))
        wd_pool = ctx.enter_context(tc.tile_pool(name="wdp", bufs=10))
        ep_pool = ctx.enter_context(tc.tile_pool(name="ep", bufs=4))
        ps_pool = ctx.enter_context(tc.tile_pool(name="ps", bufs=1, space="PSUM"))

        # ---- persistent SBUF tiles ----
        qT, kT, vpt = {}, {}, {}
        for pr in range(NPAIR):
            qT[pr] = singles.tile([P, S], BF16, name=f"qT{pr}")
            kT[pr] = singles.tile([P, S], BF16, name=f"kT{pr}")
        for h in range(HPC):
            vpt[h] = singles.tile([P, NKT, DK + 1], BF16, name=f"vph{h}")
        keep_sb = singles.tile([P, NQ, NKT, QA], BF16, name="keep_sb")
        madd_sb = singles.tile([P, NQ, NKT, QD], BF16, name="madd_sb")
        dummy = singles.tile([1, 2], F32, name="dummy")
        nc.gpsimd.memset(dummy, 0.0)

        # ---- input DMAs, in consumption order, spread over queues ----
        # iteration is (qc-block, pr, kt): the whole first block (64 slots)
        # uses only qc0 masks, qc0 columns of qT, all of kT, all vp.
        # act-ring DMAs are prologue-only (its sequencer idles pre-exp).
        nc.scalar.dma_start(out=qT[0][:, 0:QCH], in_=qkt_ext[0, 0, :, 0:QCH])
        # warm the ACT exp table while the DMAs fill
        nc.scalar.activation(
            dummy, dummy, mybir.ActivationFunctionType.Exp, scale=1.0
        )
        nc.scalar.dma_start(out=keep_sb[:, 0, 0:4], in_=keep_ext[:, 0, 0:4])
        nc.scalar.dma_start(out=keep_sb[:, 0, 4:8], in_=keep_ext[:, 0, 4:8])
        # sync ring: interleave kT0 / qc0 madd slices in consumption order
        # (slot k of block 0 needs kT0[:, 128k:...] and madd[qc0, k])
        nc.sync.dma_start(out=kT[0][:, 0:P], in_=qkt_ext[0, 1, :, 0:P])
        nc.sync.dma_start(out=madd_sb[:, 0, 0:2], in_=madd_ext[:, 0, 0:2])
        nc.sync.dma_start(out=kT[0][:, P : 4 * P], in_=qkt_ext[0, 1, :, P : 4 * P])
        nc.sync.dma_start(out=madd_sb[:, 0, 2:6], in_=madd_ext[:, 0, 2:6])
        nc.sync.dma_start(out=kT[0][:, 4 * P : 8 * P], in_=qkt_ext[0, 1, :, 4 * P : 8 * P])
        nc.sync.dma_start(out=madd_sb[:, 0, 6:NKT], in_=madd_ext[:, 0, 6:NKT])
        nc.sync.dma_start(out=kT[0][:, 8 * P : S], in_=qkt_ext[0, 1, :, 8 * P : S])
        nc.sync.dma_start(out=keep_sb[:, 0, 8:12], in_=keep_ext[:, 0, 8:12])
        nc.sync.dma_start(out=keep_sb[:, 0, 12:NKT], in_=keep_ext[:, 0, 12:NKT])
        for pr in (1, 2, 3):
            nc.sync.dma_start(out=kT[pr], in_=qkt_ext[pr, 1])
            nc.sync.dma_start(out=qT[pr][:, 0:QCH], in_=qkt_ext[pr, 0, :, 0:QCH])
        # later q-chunks + masks per qc block, in consumption order
        for qc in range(1, NQ):
            q0 = qc * QCH
            for pr in range(NPAIR):
                nc.sync.dma_start(
                    out=qT[pr][:, q0 : q0 + QCH],
                    in_=qkt_ext[pr, 0, :, q0 : q0 + QCH],
                )
            nc.sync.dma_start(out=madd_sb[:, qc], in_=madd_ext[:, qc])
            nc.sync.dma_start(out=keep_sb[:, qc, 0:8], in_=keep_ext[:, qc, 0:8])
            nc.sync.dma_start(out=keep_sb[:, qc, 8:NKT], in_=keep_ext[:, qc, 8:NKT])
        # vp on the gpsimd SWDGE queue, per head pair (pair pr from slot 16*pr)
        nc.gpsimd.dma_start(out=vpt[0][:, 0:4], in_=vp_ext[0, :, 0:4])
        nc.gpsimd.dma_start(out=vpt[1][:, 0:4], in_=vp_ext[1, :, 0:4])
        nc.gpsimd.dma_start(out=vpt[0][:, 4:NKT], in_=vp_ext[0, :, 4:NKT])
        nc.gpsimd.dma_start(out=vpt[1][:, 4:NKT], in_=vp_ext[1, :, 4:NKT])
        for h in (2, 3, 4, 5, 6, 7):
            nc.gpsimd.dma_start(out=vpt[h], in_=vp_ext[h])

        iters = [
            (pr, qc, kt)
            for qc in range(NQ)
            for pr in range(NPAIR)
            for kt in range(NKT)
        ]
        NSLOT = len(iters)  # 256

        st = {"acc": None, "pend": [], "sc": {}}

        def issue_qk(i):
            pr, qc, kt = iters[i]
            q0, k0 = qc * QCH, kt * P
            sc = ps_pool.tile(
                [P, 2, QCH], F32, tag=f"sc{i % 3}", name=f"sc_{i}", bufs=1
            )
            st["sc"][i] = sc
            nc.tensor.matmul(
                sc[:, 0],
                kT[pr][0:DK, k0 : k0 + P],
                qT[pr][0:DK, q0 : q0 + QCH],
                start=True, stop=True, tile_position=(0, 0),
            )
            nc.tensor.matmul(
                sc[:, 1],
                kT[pr][DK : 2 * DK, k0 : k0 + P],
                qT[pr][DK : 2 * DK, q0 : q0 + QCH],
                start=True, stop=True, tile_position=(64, 0),
            )

        def issue_pv(ent):
            kt, pr, qc, wa, wd, acc, vA, vB = ent
            st_, sp = (kt == 0), (kt == NKT - 1)
            # acc is [65, 2, 512]: head h confined to its own PSUM bank.
            # start=True clears has_written for the WHOLE bank, so only the
            # first piece may use it; the D piece's first write then lands on
            # cleared bits, which means overwrite -- a free region reset.
            # long streams first so the next LDWEIGHTS hides under them
            nc.tensor.matmul(
                acc[:, 0, 0:QA], vA[:, kt], wa[:, 0], start=st_, stop=sp
            )
            nc.tensor.matmul(
                acc[:, 1, 0:QA], vB[:, kt], wa[:, 1], start=st_, stop=sp
            )
            nc.tensor.matmul(
                acc[:, 0, QA:QCH], vA[:, kt], wd[:, 0].bitcast(BF16),
                start=False, stop=sp, skip_group_check=True,
            )
            nc.tensor.matmul(
                acc[:, 1, QA:QCH], vB[:, kt], wd[:, 1].bitcast(BF16),
                start=False, stop=sp, skip_group_check=True,
            )
            if kt == NKT - 1:
                # (pr, qc) complete: stage both heads' accs in SBUF with one
                # scalar-engine copy (frees the banks), DMA out; host divides.
                accS = ep_pool.tile(
                    [DK + 1, 2, QCH], F32, tag="accS", name=f"accS_{pr}_{qc}"
                )
                if pr == NPAIR - 1 and qc == NQ - 1:
                    # last block: split the copy + DMA so the tail overlaps
                    nc.scalar.copy(accS[:, 0], acc[:, 0])
                    nc.sync.dma_start(
                        out=out_ext[pr, qc, :, 0], in_=accS[:, 0]
                    )
                    nc.scalar.copy(accS[:, 1], acc[:, 1])
                    nc.scalar.dma_start(
                        out=out_ext[pr, qc, :, 1], in_=accS[:, 1]
                    )
                else:
                    nc.scalar.copy(accS, acc)
                    nc.sync.dma_start(out=out_ext[pr, qc], in_=accS)

        def run_slot(s):
            pr, qc, kt = iters[s]
            if kt == 0:
                st["acc"] = ps_pool.tile(
                    [DK + 1, 2, QCH], F32, tag="accAB",
                    name=f"acc_{pr}_{qc}", bufs=1,
                )
            if s % 2 == 0:
                # ACT-part weights for this slot pair: [p, slot, head, QA]
                st["wa"] = w_pool.tile(
                    [P, 2, 2, QA], BF16, tag="wa", name=f"wa_{s}"
                )
            wd = wd_pool.tile([P, 2, QD], U16, tag="wd", name=f"wd_{s}")
            # lagged PV pop: strictly one per slot in steady state
            npop = 2 if s >= NSLOT - 8 else 1
            for _ in range(npop):
                if len(st["pend"]) <= PV_LAG:
                    break
                issue_pv(st["pend"].pop(0))

            sc = st["sc"].pop(s)
            nc.scalar.activation(
                st["wa"][:, s % 2],
                sc[:, :, 0:QA],
                mybir.ActivationFunctionType.Exp,
                scale=0.125,
            )
            madd_ap = (
                madd_sb[:, qc, kt, :].unsqueeze(1).to_broadcast([P, 2, QD])
            )
            nc.vector.scalar_tensor_tensor(
                wd,
                sc[:, :, QA:QCH],
                SCHRAU_SCALE,
                madd_ap,
                mybir.AluOpType.mult,
                mybir.AluOpType.add,
            )
            if s + 2 < NSLOT:
                issue_qk(s + 2)

            if s % 2 == 1:
                # multiplicative keep mask over BOTH slots' ACT columns
                keep4 = (
                    keep_sb[:, qc, kt - 1 : kt + 1, :]
                    .unsqueeze(2)
                    .to_broadcast([P, 2, 2, QA])
                )
                nc.vector.tensor_mul(st["wa"], st["wa"], keep4)
                hA, hB = 2 * pr, 2 * pr + 1
                st["pend"].append(
                    (kt - 1, pr, qc, st["wa"][:, 0], st["wd_even"],
                     st["acc"], vpt[hA], vpt[hB])
                )
                st["pend"].append(
                    (kt, pr, qc, st["wa"][:, 1], wd,
                     st["acc"], vpt[hA], vpt[hB])
                )
            else:
                st["wd_even"] = wd
        issue_qk(0)
        issue_qk(1)
        for s in range(NSLOT):
            run_slot(s)
        while st["pend"]:
            issue_pv(st["pend"].pop(0))
    nc.finalize()
    return nc


_NC_CACHE = {}


def get_nc():
    if "nc" not in _NC_CACHE:
        _NC_CACHE["nc"] = build_nc()
    return _NC_CACHE["nc"]


def kernel(Q, K, V, mask, _trace=False, _tmpdir=None):
    Q = np.asarray(Q, dtype=np.float32)
    K = np.asarray(K, dtype=np.float32)
    V = np.asarray(V, dtype=np.float32)
    mask = np.asarray(mask)

    in_maps = []
    for c in range(N_CORES):
        b, h0 = c // 2, (c % 2) * HPC
        # [pair, {q,k}, 128, S]: partitions 0:64 = head A dims, 64:128 = head B
        qkt = np.empty((NPAIR, 2, P, S), BF)
        qt = Q[b, h0 : h0 + HPC].transpose(0, 2, 1).reshape(NPAIR, 2 * DK, S)
        kt = K[b, h0 : h0 + HPC].transpose(0, 2, 1).reshape(NPAIR, 2 * DK, S)
        qkt[:, 0] = qt
        qkt[:, 1] = kt
        vp = np.empty((HPC, P, NKT, DK + 1), BF)
        vp[:, :, :, 0:DK] = (
            V[b, h0 : h0 + HPC].reshape(HPC, NKT, P, DK).transpose(0, 2, 1, 3)
        )
        vp[:, :, :, DK] = 1.0
        if c % 2 == 0:
            # kp[k, q] = not mask[q, k]; layout [p, qc, kt, q-within-chunk]
            kp = (~mask[b, 0]).T.reshape(NKT, P, NQ, QCH).transpose(1, 2, 0, 3)
            keepa = np.ascontiguousarray(kp[:, :, :, 0:QA]).astype(BF)
            madd = np.where(
                kp[:, :, :, QA:QCH], MADD_KEEP, MADD_MASK
            ).astype(BF)
            madd = np.ascontiguousarray(madd)
        in_maps.append({"qkt": qkt, "vp": vp, "keepa": keepa, "madd": madd})

    nc = get_nc()
    res = run_bass_kernel_spmd(
        nc, in_maps, core_ids=list(range(N_CORES)), trace=_trace, tmpdir=_tmpdir
    )
    out = np.empty((B, H, S, DK), np.float32)
    for c in range(N_CORES):
        b, h0 = c // 2, (c % 2) * HPC
        acc = np.asarray(res.results[c]["outT"])  # [pr, qc, DK+1, 2, QCH]
        num = acc[:, :, 0:DK]                     # [pr, qc, d, h2, q]
        den = acc[:, :, DK]                       # [pr, qc, h2, q]
        o = num / den[:, :, None, :, :]
        # [pr, qc, d, h2, q] -> [pr, h2, qc, q, d] -> [h, q_all, d]
        out[b, h0 : h0 + HPC] = (
            o.transpose(0, 3, 1, 4, 2).reshape(HPC, S, DK)
        )
    if _trace:
        return out, res
    return out


# revision 83
# speedup vs baseline: 1.1883x; 1.0071x over previous
"""Sharded multi-head attention for TRN2 (8 NeuronCores).

Problem: B=4, H=16, S=2048, DK=64 attention with boolean mask [B,1,S,S]
(True entries masked out).  The 64 (batch, head) pairs are independent:
core c handles batch c//2, heads (c%2)*8 .. (c%2)*8+8.

Design (v2, 343us -> 252us): single stream, the exp is split between the
scalar and vector engines, and softmax normalization happens on the HOST.
All three compute engines run ~85% busy; the scalar engine's exp
throughput (1 elem/cycle/lane @1.2GHz, ~260ns fixed cost per ACTIVATE)
is the fundamental limiter.

  - Iteration (qc-block, pr, kt): one slot = scores sc [128, 2 heads,
    512 q] f32 in PSUM (k on partitions).  Masks are reused by all 4
    head-pairs inside a 64-slot block, so the early DMA burst is 4x
    smaller than a pr-outer order.
  - exp split by q column: the first QA=352 columns of each head go
    through the scalar engine (exact exp, then a multiplicative bf16
    keep-mask tensor_tensor on DVE at 2x over both slots of a pair);
    the remaining QD=160 go through ONE DVE scalar_tensor_tensor:
    u16 = sat_u16(round(s * (128/ln2)/8 + maskadd)), maskadd = +16256
    (=127<<7, bf16-exact) kept / -65536 masked.  The u16 bits viewed
    as bf16 ARE exp(s/8) (Schraudolph); negative saturation makes
    masked weights exactly +0.0, fusing the mask into the same pass.
    The bit-trick's sawtooth error is common-mode-cancelled by softmax
    (measured end-to-end 9.1e-3 vs 3.4e-3 all-exact; gate is 2e-2).
  - PSUM: sc rotates over THREE 2-bank tags (QK is issued 2 slots
    ahead and never waits on the exp of the tile it replaces) + one
    2-bank acc [65, 2 heads, 512] (each head's f32 row fits one bank
    exactly, so matmul start=True bank-clears stay confined).
  - PV per (kt, head) is two matmuls (ACT-part columns from the bf16
    wa tile, DVE-part columns from the u16 wd tile bitcast to bf16)
    accumulating into disjoint column ranges of the same acc bank;
    only the first piece uses start=True (a start clears has_written
    for the whole bank; the second piece's first write lands on
    cleared bits = overwrite, a free region reset).  Row 64
    accumulates the softmax denominator via the ones column in vp.
    Pops lag issue by 4+ slots so PV never waits on the DVE mask op,
    and pops are issued before the slot's exp so the per-block acc
    staging copy (scalar engine) jumps ahead of it.
  - When a (pr, qc) finishes, acc is copied PSUM->SBUF (scalar
    engine; DMA cannot touch PSUM) and DMA'd out raw; the host does
    num/den.  No on-device reciprocal/normalize at all.
  - Separate wa/wd tiles per slot-pair/slot: writes from different
    engines never share a tile (the dependency tracker is
    tile-granular; sharing serializes the writers).
  - DMAs: inputs on the sync + scalar(prologue-only) HWDGE rings in
    exact consumption order; vp on the gpsimd SWDGE queue; outputs on
    sync (the last block's output is split across sync + scalar rings
    to shorten the tail).
"""

import numpy as np
import ml_dtypes
from contextlib import ExitStack

import concourse.tile as tile
from concourse import bacc, mybir
from concourse.bass_utils import run_bass_kernel_spmd

B, H, S, DK = 4, 16, 2048, 64
N_CORES = 8
HPC = (B * H) // N_CORES  # heads per core = 8
NPAIR = HPC // 2

P = 128            # k-tile size / partition count
NKT = S // P       # 16 k tiles
QCH = 512          # q chunk per head (pair tile = [128, 1024] = 2 PSUM banks)
NQ = S // QCH      # 4 q chunks

QA = 352           # q columns per head through scalar-engine exp
QD = QCH - QA      # q columns per head through DVE bit-trick exp

BF16 = mybir.dt.bfloat16
F32 = mybir.dt.float32
U16 = mybir.dt.uint16
BF = ml_dtypes.bfloat16

LN2 = float(np.log(2.0))
SCHRAU_SCALE = 128.0 / LN2 * 0.125   # folds the 1/sqrt(dk)=1/8 score scale
MADD_KEEP = 127.0 * 128.0            # 16256, exactly representable in bf16
MADD_MASK = -65536.0                 # saturates u16 convert to 0 -> +0.0 bf16

PV_LAG = 5  # pops trail far enough that PV never waits on the DVE mask op


def build_nc():
    nc = bacc.Bacc(None, target_bir_lowering=False)
    # qkt[pair, 0] = [Q_A^T ; Q_B^T] stacked on partitions, [pair, 1] = K
    qkt_ext = nc.declare_dram_parameter("qkt", [NPAIR, 2, P, S], BF16, isOutput=False)
    # vp[h, p, t, :] = [V[h, t*128+p, :], 1.0]
    vp_ext = nc.declare_dram_parameter("vp", [HPC, P, NKT, DK + 1], BF16, isOutput=False)
    # keep_act[p, qc, t, 0:QA] = not mask[qc*512+j, t*128+p]   (ACT part)
    keep_ext = nc.declare_dram_parameter("keepa", [P, NQ, NKT, QA], BF16, isOutput=False)
    # maskadd[p, qc, t, 0:QD]: +16256 kept / -65536 masked       (DVE part)
    madd_ext = nc.declare_dram_parameter("madd", [P, NQ, NKT, QD], BF16, isOutput=False)
    # raw acc dump: [pr, qc, 65, 2, 512]; host computes rows0:64 / row64
    out_ext = nc.declare_dram_parameter(
        "outT", [NPAIR, NQ, DK + 1, 2, QCH], F32, isOutput=True
    )

    with tile.TileContext(nc) as tc, ExitStack() as ctx:
        singles = ctx.enter_context(tc.tile_pool(name="singles", bufs=1))
        w_pool = ctx.enter_context(tc.tile_pool(name="wp", bufs=8))
        wd_pool = ctx.enter_context(tc.tile_pool(name="wdp", bufs=10))
        ep_pool = ctx.enter_context(tc.tile_pool(name="ep", bufs=4))
        ps_pool = ctx.enter_context(tc.tile_pool(name="ps", bufs=1, space="PSUM"))

        # ---- persistent SBUF tiles ----
        qT, kT, vpt = {}, {}, {}
        for pr in range(NPAIR):
            qT[pr] = singles.tile([P, S], BF16, name=f"qT{pr}")
            kT[pr] = singles.tile([P, S], BF16, name=f"kT{pr}")
        for h in range(HPC):
            vpt[h] = singles.tile([P, NKT, DK + 1], BF16, name=f"vph{h}")
        keep_sb = singles.tile([P, NQ, NKT, QA], BF16, name="keep_sb")
        madd_sb = singles.tile([P, NQ, NKT, QD], BF16, name="madd_sb")
        dummy = singles.tile([1, 2], F32, name="dummy")
        nc.gpsimd.memset(dummy, 0.0)

        # ---- input DMAs, in consumption order, spread over queues ----
        # iteration is (qc-block, pr, kt): the whole first block (64 slots)
        # uses only qc0 masks, qc0 columns of qT, all of kT, all vp.
        # act-ring DMAs are prologue-only (its sequencer idles pre-exp).
        nc.scalar.dma_start(out=qT[0][:, 0:QCH], in_=qkt_ext[0, 0, :, 0:QCH])
        # warm the ACT exp table while the DMAs fill
        nc.scalar.activation(
            dummy, dummy, mybir.ActivationFunctionType.Exp, scale=1.0
        )
        nc.scalar.dma_start(out=keep_sb[:, 0, 0:4], in_=keep_ext[:, 0, 0:4])
        nc.scalar.dma_start(out=keep_sb[:, 0, 4:8], in_=keep_ext[:, 0, 4:8])
        # sync ring: interleave kT0 / qc0 madd slices in consumption order
        # (slot k of block 0 needs kT0[:, 128k:...] and madd[qc0, k])
        nc.sync.dma_start(out=kT[0][:, 0:P], in_=qkt_ext[0, 1, :, 0:P])
        nc.sync.dma_start(out=madd_sb[:, 0, 0:2], in_=madd_ext[:, 0, 0:2])
        nc.sync.dma_start(out=kT[0][:, P : 4 * P], in_=qkt_ext[0, 1, :, P : 4 * P])
        nc.sync.dma_start(out=madd_sb[:, 0, 2:6], in_=madd_ext[:, 0, 2:6])
        nc.sync.dma_start(out=kT[0][:, 4 * P : 8 * P], in_=qkt_ext[0, 1, :, 4 * P : 8 * P])
        nc.sync.dma_start(out=madd_sb[:, 0, 6:NKT], in_=madd_ext[:, 0, 6:NKT])
        nc.sync.dma_start(out=kT[0][:, 8 * P : S], in_=qkt_ext[0, 1, :, 8 * P : S])
        nc.sync.dma_start(out=keep_sb[:, 0, 8:12], in_=keep_ext[:, 0, 8:12])
        nc.sync.dma_start(out=keep_sb[:, 0, 12:NKT], in_=keep_ext[:, 0, 12:NKT])
        for pr in (1, 2, 3):
            nc.sync.dma_start(out=kT[pr], in_=qkt_ext[pr, 1])
            nc.sync.dma_start(out=qT[pr][:, 0:QCH], in_=qkt_ext[pr, 0, :, 0:QCH])
        # later q-chunks + masks per qc block, in consumption order
        for qc in range(1, NQ):
            q0 = qc * QCH
            for pr in range(NPAIR):
                nc.sync.dma_start(
                    out=qT[pr][:, q0 : q0 + QCH],
                    in_=qkt_ext[pr, 0, :, q0 : q0 + QCH],
                )
            nc.sync.dma_start(out=madd_sb[:, qc], in_=madd_ext[:, qc])
            nc.sync.dma_start(out=keep_sb[:, qc, 0:8], in_=keep_ext[:, qc, 0:8])
            nc.sync.dma_start(out=keep_sb[:, qc, 8:NKT], in_=keep_ext[:, qc, 8:NKT])
        # vp on the gpsimd SWDGE queue, per head pair (pair pr from slot 16*pr)
        nc.gpsimd.dma_start(out=vpt[0][:, 0:4], in_=vp_ext[0, :, 0:4])
        nc.gpsimd.dma_start(out=vpt[1][:, 0:4], in_=vp_ext[1, :, 0:4])
        nc.gpsimd.dma_start(out=vpt[0][:, 4:NKT], in_=vp_ext[0, :, 4:NKT])
        nc.gpsimd.dma_start(out=vpt[1][:, 4:NKT], in_=vp_ext[1, :, 4:NKT])
        for h in (2, 3, 4, 5, 6, 7):
            nc.gpsimd.dma_start(out=vpt[h], in_=vp_ext[h])

        iters = [
            (pr, qc, kt)
            for qc in range(NQ)
            for pr in range(NPAIR)
            for kt in range(NKT)
        ]
        NSLOT = len(iters)  # 256

        st = {"acc": None, "pend": [], "sc": {}}

        def issue_qk(i):
            pr, qc, kt = iters[i]
            q0, k0 = qc * QCH, kt * P
            sc = ps_pool.tile(
                [P, 2, QCH], F32, tag=f"sc{i % 3}", name=f"sc_{i}", bufs=1
            )
            st["sc"][i] = sc
            nc.tensor.matmul(
                sc[:, 0],
                kT[pr][0:DK, k0 : k0 + P],
                qT[pr][0:DK, q0 : q0 + QCH],
                start=True, stop=True, tile_position=(0, 0),
            )
            nc.tensor.matmul(
                sc[:, 1],
                kT[pr][DK : 2 * DK, k0 : k0 + P],
                qT[pr][DK : 2 * DK, q0 : q0 + QCH],
                start=True, stop=True, tile_position=(64, 0),
            )

        def issue_pv(ent):
            kt, pr, qc, wa, wd, acc, vA, vB = ent
            st_, sp = (kt == 0), (kt == NKT - 1)
            # acc is [65, 2, 512]: head h confined to its own PSUM bank.
            # start=True clears has_written for the WHOLE bank, so only the
            # first piece may use it; the D piece's first write then lands on
            # cleared bits, which means overwrite -- a free region reset.
            # long streams first so the next LDWEIGHTS hides under them
            nc.tensor.matmul(
                acc[:, 0, 0:QA], vA[:, kt], wa[:, 0], start=st_, stop=sp
            )
            nc.tensor.matmul(
                acc[:, 1, 0:QA], vB[:, kt], wa[:, 1], start=st_, stop=sp
            )
            nc.tensor.matmul(
                acc[:, 0, QA:QCH], vA[:, kt], wd[:, 0].bitcast(BF16),
                start=False, stop=sp, skip_group_check=True,
            )
            nc.tensor.matmul(
                acc[:, 1, QA:QCH], vB[:, kt], wd[:, 1].bitcast(BF16),
                start=False, stop=sp, skip_group_check=True,
            )
            if kt == NKT - 1:
                # (pr, qc) complete: stage both heads' accs in SBUF with one
                # scalar-engine copy (frees the banks), DMA out; host divides.
                accS = ep_pool.tile(
                    [DK + 1, 2, QCH], F32, tag="accS", name=f"accS_{pr}_{qc}"
                )
                if pr == NPAIR - 1 and qc == NQ - 1:
                    # last block: split the copy + DMA so the tail overlaps
                    nc.scalar.copy(accS[:, 0], acc[:, 0])
                    nc.sync.dma_start(
                        out=out_ext[pr, qc, :, 0], in_=accS[:, 0]
                    )
                    nc.scalar.copy(accS[:, 1], acc[:, 1])
                    nc.scalar.dma_start(
                        out=out_ext[pr, qc, :, 1], in_=accS[:, 1]
                    )
                else:
                    nc.scalar.copy(accS, acc)
                    nc.sync.dma_start(out=out_ext[pr, qc], in_=accS)

        def run_slot(s):
            pr, qc, kt = iters[s]
            if kt == 0:
                st["acc"] = ps_pool.tile(
                    [DK + 1, 2, QCH], F32, tag="accAB",
                    name=f"acc_{pr}_{qc}", bufs=1,
                )
            if s % 2 == 0:
                # ACT-part weights for this slot pair: [p, slot, head, QA]
                st["wa"] = w_pool.tile(
                    [P, 2, 2, QA], BF16, tag="wa", name=f"wa_{s}"
                )
            wd = wd_pool.tile([P, 2, QD], U16, tag="wd", name=f"wd_{s}")
            # lagged PV pop: strictly one per slot in steady state
            npop = 2 if s >= NSLOT - 8 else 1
            for _ in range(npop):
                if len(st["pend"]) <= PV_LAG:
                    break
                issue_pv(st["pend"].pop(0))

            sc = st["sc"].pop(s)
            nc.scalar.activation(
                st["wa"][:, s % 2],
                sc[:, :, 0:QA],
                mybir.ActivationFunctionType.Exp,
                scale=0.125,
            )
            madd_ap = (
                madd_sb[:, qc, kt, :].unsqueeze(1).to_broadcast([P, 2, QD])
            )
            nc.vector.scalar_tensor_tensor(
                wd,
                sc[:, :, QA:QCH],
                SCHRAU_SCALE,
                madd_ap,
                mybir.AluOpType.mult,
                mybir.AluOpType.add,
            )
            if s + 2 < NSLOT:
                issue_qk(s + 2)

            if s % 2 == 1:
                # multiplicative keep mask over BOTH slots' ACT columns
                keep4 = (
                    keep_sb[:, qc, kt - 1 : kt + 1, :]
                    .unsqueeze(2)
                    .to_broadcast([P, 2, 2, QA])
                )
                nc.vector.tensor_mul(st["wa"], st["wa"], keep4)
                hA, hB = 2 * pr, 2 * pr + 1
                st["pend"].append(
                    (kt - 1, pr, qc, st["wa"][:, 0], st["wd_even"],
                     st["acc"], vpt[hA], vpt[hB])
                )
                st["pend"].append(
                    (kt, pr, qc, st["wa"][:, 1], wd,
                     st["acc"], vpt[hA], vpt[hB])
                )
            else:
                st["wd_even"] = wd
        issue_qk(0)
        issue_qk(1)
        for s in range(NSLOT):
            run_slot(s)
        while st["pend"]:
            issue_pv(st["pend"].pop(0))
    nc.finalize()
    return nc


_NC_CACHE = {}


def get_nc():
    if "nc" not in _NC_CACHE:
        _NC_CACHE["nc"] = build_nc()
    return _NC_CACHE["nc"]


def kernel(Q, K, V, mask, _trace=False, _tmpdir=None):
    Q = np.asarray(Q, dtype=np.float32)
    K = np.asarray(K, dtype=np.float32)
    V = np.asarray(V, dtype=np.float32)
    mask = np.asarray(mask)

    in_maps = []
    for c in range(N_CORES):
        b, h0 = c // 2, (c % 2) * HPC
        # [pair, {q,k}, 128, S]: partitions 0:64 = head A dims, 64:128 = head B
        qkt = np.empty((NPAIR, 2, P, S), BF)
        qt = Q[b, h0 : h0 + HPC].transpose(0, 2, 1).reshape(NPAIR, 2 * DK, S)
        kt = K[b, h0 : h0 + HPC].transpose(0, 2, 1).reshape(NPAIR, 2 * DK, S)
        qkt[:, 0] = qt
        qkt[:, 1] = kt
        vp = np.empty((HPC, P, NKT, DK + 1), BF)
        vp[:, :, :, 0:DK] = (
            V[b, h0 : h0 + HPC].reshape(HPC, NKT, P, DK).transpose(0, 2, 1, 3)
        )
        vp[:, :, :, DK] = 1.0
        if c % 2 == 0:
            # kp[k, q] = not mask[q, k]; layout [p, qc, kt, q-within-chunk]
            kp = (~mask[b, 0]).T.reshape(NKT, P, NQ, QCH).transpose(1, 2, 0, 3)
            keepa = np.ascontiguousarray(kp[:, :, :, 0:QA]).astype(BF)
            madd = np.where(
                kp[:, :, :, QA:QCH], MADD_KEEP, MADD_MASK
            ).astype(BF)
            madd = np.ascontiguousarray(madd)
        in_maps.append({"qkt": qkt, "vp": vp, "keepa": keepa, "madd": madd})

    nc = get_nc()
    res = run_bass_kernel_spmd(
        nc, in_maps, core_ids=list(range(N_CORES)), trace=_trace, tmpdir=_tmpdir
    )
    out = np.empty((B, H, S, DK), np.float32)
    for c in range(N_CORES):
        b, h0 = c // 2, (c % 2) * HPC
        acc = np.asarray(res.results[c]["outT"])  # [pr, qc, DK+1, 2, QCH]
        num = acc[:, :, 0:DK]                     # [pr, qc, d, h2, q]
        den = acc[:, :, DK]                       # [pr, qc, h2, q]
        o = num / den[:, :, None, :, :]
        # [pr, qc, d, h2, q] -> [pr, h2, qc, q, d] -> [h, q_all, d]
        out[b, h0 : h0 + HPC] = (
            o.transpose(0, 3, 1, 4, 2).reshape(HPC, S, DK)
        )
    if _trace:
        return out, res
    return out
